# revision 26
# baseline (speedup 1.0000x reference)
"""BayesianKalmanNet Trainium2 kernel.

Sharding: data-parallel over batch (128 -> 16 per core x 8 cores); the
num_samples=16 ensemble stays on-core so the per-core GRU matmul free dim is
16*16 = 256.  Activations live transposed ([HID, b*NS+j]) so the GRU
recurrence needs no transposes.  Matmuls run as float32r (FP22), full-rate at
N=256.  Dropout masks are reproduced bit-exactly on the host (jax CPU
threefry) and shipped as uint8 with the 1/keep scale folded into weights /
activation scales.  The device emits only ens_v = K @ innov per timestep; the
host reconstructs x_filt and P_filt.

Schedule: the hidden state is double-buffered so h-updates retire per M-tile;
r/z/gi_n/n-combine/h-update are interleaved per M-tile; gh_n for step t+1 is
computed at the end of step t (it only needs h(t)) and drained to SBUF so the
PE has useful work covering the serial inter-step tail.
"""

import os
import functools

import numpy as np

os.environ.setdefault("JAX_COMPILATION_CACHE_DIR", "/tmp/jaxcache")
os.environ.setdefault("JAX_PERSISTENT_CACHE_MIN_COMPILE_TIME_SECS", "2")

import concourse.bass as bass
import concourse.tile as tile
from concourse import bacc, mybir
from concourse.bass import ts, ds
from concourse.bass_utils import run_bass_kernel_spmd

F32 = mybir.dt.float32
F32R = mybir.dt.float32r
I32 = mybir.dt.int32
U8 = mybir.dt.uint8
AL = mybir.AluOpType
AF = mybir.ActivationFunctionType

NCORES = 8
B, T, S, O = 128, 32, 8, 4
HID = 800
NS = 16
BL = B // NCORES          # 16 batches per core
JB = BL * NS              # 256 = free dim
SO = S * O                # 32
KEEP = 0.8
INV_KEEP = 1.25
MAGIC = 0x5F3759DF

KT_MAIN = 6
K_TAIL = HID - KT_MAIN * 128   # 32
M_TILES = [(i * 128, min(128, HID - i * 128)) for i in range(7)]

MM_PHASE = {}
RZ_BUFS = 3
GHN_BUFS = 2
SIN_POOL = False


def _r3(ap):
    """[P, 256] -> [P, 16, 16] (b outer, j inner)."""
    return ap.rearrange("p (b j) -> p b j", j=NS)


@functools.lru_cache(maxsize=4)
def _build_nc(n_steps=T, repeat=1):
    MM_PHASE.clear()
    nc = bacc.Bacc("TRN2", target_bir_lowering=False, debug=False,
                   num_devices=NCORES)

    d_wrz = nc.dram_tensor("wrz", [128, 13 * 1600], F32R, kind="ExternalInput")
    d_win = nc.dram_tensor("win", [128, 7 * HID], F32R, kind="ExternalInput")
    d_whn = nc.dram_tensor("whn", [128, 7 * HID], F32R, kind="ExternalInput")
    d_w1t = nc.dram_tensor("w1t", [17, HID], F32, kind="ExternalInput")
    d_wout = nc.dram_tensor("wout", [128, 7 * SO], F32R, kind="ExternalInput")
    d_bias = nc.dram_tensor("biases", [128, 36], F32, kind="ExternalInput")
    d_selt = nc.dram_tensor("selt", [SO, S], F32, kind="ExternalInput")
    d_et = nc.dram_tensor("et", [O, SO], F32, kind="ExternalInput")
    d_yb = nc.dram_tensor("yb", [BL, n_steps * O], F32, kind="ExternalInput")
    d_m1m = nc.dram_tensor("m1m", [n_steps, 128, KT_MAIN * JB], U8,
                           kind="ExternalInput")
    d_m1t = nc.dram_tensor("m1t", [n_steps, K_TAIL, JB], U8,
                           kind="ExternalInput")
    d_m2 = nc.dram_tensor("m2", [n_steps, SO, JB], U8, kind="ExternalInput")
    d_ens = nc.dram_tensor("ens", [n_steps, S, JB], F32, kind="ExternalOutput")

    def _mm(phase, *a, **k):
        bi = nc.tensor.matmul(*a, **k)
        MM_PHASE[bi.ins.name] = phase
        return bi

    with tile.TileContext(nc) as tc:
        with tc.tile_pool(name="const", bufs=1) as cpool, \
             tc.tile_pool(name="work", bufs=2) as wpool, \
             tc.tile_pool(name="ps", bufs=2, space="PSUM") as pspool:

            # ---- persistent tiles + weight loads ----
            bias_sb = cpool.tile([128, 36], F32, tag="bias")
            nc.sync.dma_start(bias_sb[:], d_bias.ap())
            selt_sb = cpool.tile([SO, S], F32, tag="selt")
            nc.sync.dma_start(selt_sb[:], d_selt.ap())
            et_sb = cpool.tile([O, SO], F32, tag="et")
            nc.sync.dma_start(et_sb[:], d_et.ap())
            w1t_sb = cpool.tile([17, HID], F32, tag="w1t")
            nc.sync.dma_start(w1t_sb[:], d_w1t.ap())
            yb_sb = cpool.tile([BL, n_steps * O], F32, tag="yb")
            nc.sync.dma_start(yb_sb[:], d_yb.ap())
            whn_sb = cpool.tile([128, 7 * HID], F32R, tag="whn")
            nc.sync.dma_start(whn_sb[:], d_whn.ap())
            wout_sb = cpool.tile([128, 7 * SO], F32R, tag="wout")
            nc.sync.dma_start(wout_sb[:], d_wout.ap())
            wrz_sb = cpool.tile([128, 13 * 1600], F32R, tag="wrz")
            nc.sync.dma_start(wrz_sb[:, 6 * 1600:], d_wrz.ap()[:, 6 * 1600:])
            nc.sync.dma_start(wrz_sb[:, 0:6 * 1600], d_wrz.ap()[:, 0:6 * 1600])
            win_sb = cpool.tile([128, 7 * HID], F32R, tag="win")
            nc.sync.dma_start(win_sb[:], d_win.ap())

            # double-buffered hidden state (read buf[t%2], write buf[1-t%2])
            # t6cat: rows 0:32 = h tail, rows 32:64 = a tail
            h_bufs, t6c_bufs = [], []
            for i in range(2):
                hb = cpool.tile([128, KT_MAIN * JB], F32R, tag=f"h{i}",
                                name=f"h{i}")
                nc.vector.memzero(hb[:])
                h_bufs.append(hb)
                tc_ = cpool.tile([2 * K_TAIL, JB], F32R, tag=f"t6c{i}",
                                 name=f"t6c{i}")
                nc.vector.memzero(tc_[:])
                t6c_bufs.append(tc_)

            tst_bufs, tstT_bufs, mst_bufs, mstT_bufs = [], [], [], []
            for i in range(2):
                tb_ = cpool.tile([32, 32], F32, tag=f"tstp{i}",
                                 name=f"tstp{i}")
                nc.vector.memset(tb_[:], 0.0)
                nc.vector.memset(tb_[0:BL, 16:17], 1.0)   # W1 bias row
                tst_bufs.append(tb_)
                tt_ = cpool.tile([32, 32], F32, tag=f"tstTp{i}",
                                 name=f"tstTp{i}")
                tstT_bufs.append(tt_)
                mb_ = cpool.tile([32, 32], F32, tag=f"mstp{i}",
                                 name=f"mstp{i}")
                nc.vector.memset(mb_[:], 0.0)
                mst_bufs.append(mb_)
                mt_ = cpool.tile([32, 32], F32, tag=f"mstTp{i}",
                                 name=f"mstTp{i}")
                mstT_bufs.append(mt_)

            xz = cpool.tile([BL, S], F32, tag="xz")
            nc.vector.memset(xz[:], 0.0)
            dz = cpool.tile([BL, S], F32, tag="dz")
            nc.vector.memset(dz[:], 0.0)
            scr8 = cpool.tile([BL, S], F32, tag="scr8")
            scr4 = cpool.tile([BL, O], F32, tag="scr4")

            def sm(tg, shape=(BL, S), dt=F32):
                return wpool.tile(list(shape), dt, tag=tg, name=tg)

            def emit_ghn(h_sb, t6c_sb):
                """gh_n for the next step: 49 MMs + ACT drain to SBUF."""
                kord_h = [(h_sb[:, ts(k, JB)], k) for k in range(KT_MAIN)]
                kord_h += [(t6c_sb[0:K_TAIL, :], 6)]
                tiles = []
                for mi, (mof, mp) in enumerate(M_TILES):
                    gh = pspool.tile([128, JB], F32, tag="ghn", name="ghnps",
                                     bufs=GHN_BUFS)
                    for ki, (rhs, col) in enumerate(kord_h):
                        _mm("gh_n", gh[0:mp, :],
                            whn_sb[0:rhs.partition_size(),
                                   ds(col * HID + mof, mp)],
                            rhs,
                            start=(ki == 0), stop=(ki == len(kord_h) - 1))
                    ghs = wpool.tile([128, JB], F32, tag="ghs", name="ghs",
                                     bufs=7)
                    nc.scalar.copy(ghs[0:mp, :], gh[0:mp, :])
                    tiles.append(ghs)
                return tiles

            x_prev, dx_prev = xz, dz
            ghs_next = emit_ghn(h_bufs[0], t6c_bufs[0])   # h == 0 prologue

            for step in range(n_steps * repeat):
                t = step % n_steps
                h_rd, h_wr = h_bufs[step % 2], h_bufs[1 - step % 2]
                t6c_rd, t6c_wr = t6c_bufs[step % 2], t6c_bufs[1 - step % 2]

                # ---- mask DMAs ----
                m1m = wpool.tile([128, KT_MAIN * JB], U8, tag="m1m",
                                 name="m1m")
                nc.sync.dma_start(m1m[:], d_m1m.ap()[t])
                m1t = wpool.tile([2 * K_TAIL, JB], U8, tag="m1t", name="m1t")
                nc.sync.dma_start(m1t[K_TAIL:2 * K_TAIL, :], d_m1t.ap()[t])
                m2t = wpool.tile([SO, JB], U8, tag="m2t", name="m2t")
                nc.sync.dma_start(m2t[:], d_m2.ap()[t])

                # ---- x_pred = 0.9 x + 0.1 sin(x) (deg-5 odd poly) ----
                se = nc.gpsimd if SIN_POOL else nc.vector
                tx = sm("tx")
                se.tensor_mul(tx[:], x_prev[:], x_prev[:])
                u1 = sm("u1")
                se.tensor_scalar(u1[:], tx[:], 1.0 / 120.0, -1.0 / 6.0,
                                 AL.mult, AL.add)
                v1 = sm("v1")
                se.tensor_mul(v1[:], u1[:], tx[:])
                sfac = sm("sfac")
                # sin(x)/x ~= 1 + v1 ; x_pred = x*(0.9 + 0.1*(1 + v1))
                se.tensor_scalar(sfac[:], v1[:], 0.1, 1.0,
                                 AL.mult, AL.add)
                x_pred = sm("x_pred")
                se.tensor_mul(x_pred[:], x_prev[:], sfac[:])

                # ---- innov into tst[:, 0:4] ----
                yhat = sm("yhat", (BL, O))
                nc.scalar.activation(yhat[:], x_pred[:, 0:O], AF.Tanh)
                tst = tst_bufs[step % 2]
                nc.vector.tensor_sub(tst[0:BL, 0:O], yb_sb[:, ts(t, O)],
                                     yhat[:])

                # ---- l2 norms on DVE; inv = rsqrt via quake + 1 Newton ----
                ss2 = sm("ss2", (BL, 2))
                nc.vector.scalar_tensor_tensor(
                    scr8[:], dx_prev[:], 1.0, dx_prev[:], AL.mult, AL.mult,
                    accum_out=ss2[:, 0:1])
                nc.vector.scalar_tensor_tensor(
                    scr4[:], tst[0:BL, 0:O], 1.0, tst[0:BL, 0:O],
                    AL.mult, AL.mult, accum_out=ss2[:, 1:2])
                sscl = sm("sscl", (BL, 2))
                nc.vector.tensor_scalar_max(sscl[:], ss2[:], 1e-24)
                qi = sm("qi", (BL, 2), I32)
                nc.vector.tensor_scalar(qi[:], sscl[:].bitcast(I32), 1, None,
                                        AL.logical_shift_right)
                qi2 = sm("qi2", (BL, 2), I32)
                nc.vector.tensor_scalar(qi2[:], qi[:], -1, MAGIC,
                                        AL.mult, AL.add)
                yv = qi2[:].bitcast(F32)
                for it in range(2):
                    nw1 = sm(f"nw1_{it}", (BL, 2))
                    nc.vector.tensor_mul(nw1[:], yv, yv)
                    nw2 = sm(f"nw2_{it}", (BL, 2))
                    nc.vector.tensor_mul(nw2[:], nw1[:], sscl[:])
                    nw3 = sm(f"nw3_{it}", (BL, 2))
                    nc.vector.tensor_scalar(nw3[:], nw2[:], -0.5, 1.5,
                                            AL.mult, AL.add)
                    ny = sm(f"ny_{it}", (BL, 2))
                    nc.vector.tensor_mul(ny[:], yv, nw3[:])
                    yv = ny[:]
                nc.vector.tensor_mul(tst[0:BL, 4:12], dx_prev[:],
                                     yv[:, 0:1].to_broadcast([BL, S]))
                nc.vector.tensor_mul(tst[0:BL, 12:16], tst[0:BL, 0:O],
                                     yv[:, 1:2].to_broadcast([BL, O]))
                tstT = tstT_bufs[step % 2]
                nc.vector.transpose(tstT[:], tst[:])

                # innov broadcast for the ens contraction (used late)
                ibc = wpool.tile([O, JB], F32, tag="ibc", name="ibc")
                nc.vector.tensor_copy(
                    _r3(ibc[:]),
                    tstT[0:O, 0:BL, None].to_broadcast([O, BL, NS]))

                # ---- a = relu(1.25*(W1 @ nn_in + b1)) (transposed) ----
                aT = wpool.tile([128, 7 * BL], F32, tag="aT", name="aT")
                aps = pspool.tile([128, 7 * BL], F32, tag="misc", name="aps", bufs=1)
                for mi, (mof, mp) in enumerate(M_TILES):
                    if mi < KT_MAIN:
                        out_ap = aps[0:mp, ts(mi, BL)]
                    else:
                        out_ap = aps[K_TAIL:2 * K_TAIL, ts(mi, BL)]
                    _mm("a_w1", out_ap,
                        w1t_sb[0:17, ds(mof, mp)], tstT[0:17, 0:BL],
                        start=True, stop=True)
                nc.scalar.activation(aT[:, 0:KT_MAIN * BL],
                                     aps[:, 0:KT_MAIN * BL], AF.Relu,
                                     scale=INV_KEEP)
                nc.scalar.activation(aT[K_TAIL:2 * K_TAIL, ds(KT_MAIN * BL, BL)],
                                     aps[K_TAIL:2 * K_TAIL, ds(KT_MAIN * BL, BL)],
                                     AF.Relu, scale=INV_KEEP)

                # ---- a_ens = aT (bcast over j) * m1 ----
                aens = wpool.tile([128, KT_MAIN * JB], F32R, tag="aens",
                                  name="aens")
                for k in range(KT_MAIN):
                    nc.vector.tensor_mul(
                        _r3(aens[:, ts(k, JB)]),
                        aT[:, ts(k, BL), None].to_broadcast([128, BL, NS]),
                        _r3(m1m[:, ts(k, JB)]))
                nc.vector.tensor_mul(
                    _r3(t6c_rd[K_TAIL:2 * K_TAIL, :]),
                    aT[K_TAIL:2 * K_TAIL, ds(6 * BL, BL), None].to_broadcast(
                        [K_TAIL, BL, NS]),
                    _r3(m1t[K_TAIL:2 * K_TAIL, :]))

                # ---- per-M-tile: r, z, gi_n, n-combine, h-update ----
                korder = [(h_rd[:, ts(k, JB)], 6 + k) for k in range(KT_MAIN)]
                korder += [(aens[:, ts(k, JB)], k) for k in range(KT_MAIN)]
                korder += [(t6c_rd[0:2 * K_TAIL, :], 12)]
                kord_a = [(aens[:, ts(k, JB)], k) for k in range(KT_MAIN)]
                kord_a += [(t6c_rd[K_TAIL:2 * K_TAIL, :], 6)]
                ghs_cur = ghs_next

                for mi, (mof, mp) in enumerate(M_TILES):
                    gate_sb = {}
                    for g, gname, bcol0 in ((0, "rT", 0), (1, "zT", 7)):
                        mbase = g * HID + mof
                        ps = pspool.tile([128, JB], F32, tag="rz",
                                         name="rzps", bufs=RZ_BUFS)
                        for ki, (rhs, col) in enumerate(korder):
                            _mm("rz_h" if ki < 7 else "rz_a",
                                ps[0:mp, :],
                                wrz_sb[0:rhs.partition_size(),
                                       ds(col * 1600 + mbase, mp)],
                                rhs,
                                start=(ki == 0), stop=(ki == len(korder) - 1))
                        gt = wpool.tile([128, JB], F32, tag=gname, name=gname,
                                        bufs=3)
                        nc.scalar.activation(
                            gt[0:mp, :], ps[0:mp, :], AF.Sigmoid,
                            bias=bias_sb[0:mp, bcol0 + mi:bcol0 + mi + 1])
                        gate_sb[gname] = gt

                    gi = pspool.tile([128, JB], F32, tag="gin", name="ginps")
                    for ki, (rhs, col) in enumerate(kord_a):
                        bp = rhs.base_partition()
                        _mm("gi_n", gi[0:mp, :],
                            win_sb[bp:bp + rhs.partition_size(),
                                   ds(col * HID + mof, mp)],
                            rhs,
                            start=(ki == 0), stop=(ki == len(kord_a) - 1))
                    tb = wpool.tile([128, JB], F32, tag="tb", name="tb")
                    nc.vector.scalar_tensor_tensor(
                        tb[0:mp, :], ghs_cur[mi][0:mp, :],
                        bias_sb[0:mp, 21 + mi:22 + mi],
                        gate_sb["rT"][0:mp, :], AL.add, AL.mult)
                    npre = wpool.tile([128, JB], F32, tag="npre", name="npre")
                    nc.vector.scalar_tensor_tensor(
                        npre[0:mp, :], gi[0:mp, :],
                        bias_sb[0:mp, 14 + mi:15 + mi], tb[0:mp, :],
                        AL.add, AL.add)
                    nT = wpool.tile([128, JB], F32, tag="nT", name="nT",
                                    bufs=3)
                    nc.scalar.activation(nT[0:mp, :], npre[0:mp, :], AF.Tanh)

                    # h_new = n + z * (h - n) into the write buffer
                    hsrc = (h_rd[:, ts(mi, JB)] if mi < KT_MAIN
                            else t6c_rd[0:K_TAIL, :])
                    htgt = (h_wr[:, ts(mi, JB)] if mi < KT_MAIN
                            else t6c_wr[0:K_TAIL, :])
                    qb = wpool.tile([128, JB], F32, tag="qb", name="qb")
                    nc.vector.scalar_tensor_tensor(
                        qb[0:mp, :], nT[0:mp, :], -1.0, hsrc,
                        AL.mult, AL.add)
                    eb = wpool.tile([128, JB], F32, tag="eb", name="eb")
                    nc.vector.tensor_mul(eb[0:mp, :],
                                         gate_sb["zT"][0:mp, :], qb[0:mp, :])
                    nc.vector.tensor_add(htgt, nT[0:mp, :], eb[0:mp, :])

                # ---- K_vec = 1.25*(W_out @ h_new + b_out) * m2 ----
                kord_hw = [(h_wr[:, ts(k, JB)], k) for k in range(KT_MAIN)]
                kord_hw += [(t6c_wr[0:K_TAIL, :], 6)]
                kv = pspool.tile([SO, JB], F32, tag="misc", name="kvps", bufs=1)
                for ki, (rhs, col) in enumerate(kord_hw):
                    _mm("kvec", kv[:, :],
                        wout_sb[0:rhs.partition_size(), ds(col * SO, SO)],
                        rhs,
                        start=(ki == 0), stop=(ki == len(kord_hw) - 1))
                KT = wpool.tile([SO, JB], F32, tag="KT", name="KT")
                nc.vector.scalar_tensor_tensor(
                    KT[:], kv[:, :], bias_sb[0:SO, 35:36], m2t[:],
                    AL.add, AL.mult)

                # ---- ens_v = Sel @ (K .* innov_exp) ----
                iexp = pspool.tile([SO, JB], F32, tag="misc", name="iexpps", bufs=1)
                _mm("iexp", iexp[:, :], et_sb[:], ibc[:],
                    start=True, stop=True)
                prod = wpool.tile([SO, JB], F32, tag="prod", name="prod")
                nc.vector.tensor_mul(prod[:], KT[:], iexp[:, :])
                ensps = pspool.tile([S, JB], F32, tag="misc", name="ensps", bufs=1)
                _mm("ens", ensps[:, :], selt_sb[:], prod[:],
                    start=True, stop=True)
                enssb = wpool.tile([S, JB], F32, tag="enssb", name="enssb")
                nc.scalar.copy(enssb[:], ensps[:, :])
                nc.sync.dma_start(d_ens.ap()[t], enssb[:])

                # ---- x_filt = x_pred + mean_j(ens_v) ----
                mtmp = sm("mtmp", (S, BL))
                nc.vector.tensor_reduce(mtmp[:], _r3(ensps[:, :]),
                                        mybir.AxisListType.X, AL.add)
                mst = mst_bufs[step % 2]
                nc.vector.tensor_scalar_mul(mst[0:S, 0:BL], mtmp[:],
                                            1.0 / NS)
                mstT = mstT_bufs[step % 2]
                nc.vector.transpose(mstT[:], mst[:])
                xfb = sm("xfb")
                nc.vector.tensor_add(xfb[:], x_pred[:], mstT[0:BL, 0:S])

                # gh_n for next step (gives PE work covering the serial tail)
                ghs_next = emit_ghn(h_wr, t6c_wr)

                x_prev = xfb
                dx_prev = mstT[0:BL, 0:S]

    nc.compile()
    return nc


def _host_inputs(y_seq, W1, b1, W_ih, W_hh, b_ih, b_hh, W_out, b_out,
                 num_samples, n_steps=T):
    """Build the 8 per-core input maps."""
    import jax

    assert int(num_samples) == NS
    y_seq = np.asarray(y_seq, np.float32)
    W1 = np.asarray(W1, np.float32)
    b1 = np.asarray(b1, np.float32)
    W_ih = np.asarray(W_ih, np.float32)
    W_hh = np.asarray(W_hh, np.float32)
    b_ih = np.asarray(b_ih, np.float32)
    b_hh = np.asarray(b_hh, np.float32)
    W_out = np.asarray(W_out, np.float32)
    b_out = np.asarray(b_out, np.float32)

    WihT = np.ascontiguousarray(W_ih.T)   # [HID, 2400]
    WhhT = np.ascontiguousarray(W_hh.T)

    wrz = np.zeros((128, 13 * 1600), np.float32)
    for k in range(6):
        wrz[:, k * 1600:(k + 1) * 1600] = WihT[k * 128:(k + 1) * 128, 0:1600]
        wrz[:, (6 + k) * 1600:(7 + k) * 1600] = \
            WhhT[k * 128:(k + 1) * 128, 0:1600]
    # merged tail block: rows 0:32 = W_hh tail (h), rows 32:64 = W_ih tail (a)
    wrz[0:K_TAIL, 12 * 1600:] = WhhT[768:HID, 0:1600]
    wrz[K_TAIL:2 * K_TAIL, 12 * 1600:] = WihT[768:HID, 0:1600]

    def pack_n(WT):
        w = np.zeros((128, 7 * HID), np.float32)
        for k in range(6):
            w[:, k * HID:(k + 1) * HID] = WT[k * 128:(k + 1) * 128, 1600:2400]
        w[0:K_TAIL, 6 * HID:] = WT[768:HID, 1600:2400]
        return w

    win = pack_n(WihT)
    win[K_TAIL:2 * K_TAIL, 6 * HID:] = win[0:K_TAIL, 6 * HID:]
    win[0:K_TAIL, 6 * HID:] = 0.0
    whn = pack_n(WhhT)

    w1t = np.zeros((17, HID), np.float32)   # rows 0:4 zero (raw innov)
    w1t[4:16] = W1.T                        # dxn rows 4:12, innovn 12:16
    w1t[16] = b1                            # bias via ones row of nn_inT
    WoutTs = np.ascontiguousarray(W_out.T) * INV_KEEP      # [800, 32]
    wout = np.zeros((128, 7 * SO), np.float32)
    for k in range(6):
        wout[:, k * SO:(k + 1) * SO] = WoutTs[k * 128:(k + 1) * 128]
    wout[0:K_TAIL, 6 * SO:] = WoutTs[768:HID]

    biases = np.zeros((128, 36), np.float32)
    b_rz = b_ih[0:1600] + b_hh[0:1600]
    for mi, (mof, mp) in enumerate(M_TILES):
        biases[0:mp, mi] = b_rz[mof:mof + mp]                    # r
        biases[0:mp, 7 + mi] = b_rz[HID + mof:HID + mof + mp]    # z
        biases[0:mp, 14 + mi] = b_ih[1600 + mof:1600 + mof + mp]
        biases[0:mp, 21 + mi] = b_hh[1600 + mof:1600 + mof + mp]
        biases[0:mp, 28 + mi] = b1[mof:mof + mp] * INV_KEEP
    biases[0:SO, 35] = b_out * INV_KEEP

    selt = np.zeros((SO, S), np.float32)
    for s in range(S):
        selt[s * O:(s + 1) * O, s] = 1.0
    et = np.zeros((O, SO), np.float32)
    for s in range(S):
        for o in range(O):
            et[o, s * O + o] = 1.0

    # dropout masks (bit-exact threefry reproduction of the reference)
    import jax.random as jr
    cpu = jax.devices("cpu")[0]
    m1_all = np.empty((n_steps, NS, B, HID), np.uint8)
    m2_all = np.empty((n_steps, NS, B, SO), np.uint8)
    with jax.default_device(cpu):
        drop_key = jr.key(42)
        for t in range(n_steps):
            k1, k2 = jr.split(jr.fold_in(drop_key, t))
            m1_all[t] = np.asarray(jr.bernoulli(k1, KEEP, (NS, B, HID)),
                                   np.uint8)
            m2_all[t] = np.asarray(jr.bernoulli(k2, KEEP, (NS, B, SO)),
                                   np.uint8)

    in_maps = []
    for c in range(NCORES):
        bs = slice(c * BL, (c + 1) * BL)
        m1T = m1_all[:, :, bs, :].transpose(0, 3, 2, 1).reshape(
            n_steps, HID, JB)
        m1m = np.ascontiguousarray(
            m1T[:, 0:768, :].reshape(n_steps, 6, 128, JB)
            .transpose(0, 2, 1, 3).reshape(n_steps, 128, 6 * JB))
        m1t = np.ascontiguousarray(m1T[:, 768:HID, :])
        m2T = np.ascontiguousarray(
            m2_all[:, :, bs, :].transpose(0, 3, 2, 1).reshape(
                n_steps, SO, JB))
        yb = np.ascontiguousarray(
            y_seq[bs, 0:n_steps].reshape(BL, n_steps * O))
        in_maps.append({
            "wrz": wrz, "win": win, "whn": whn, "w1t": w1t, "wout": wout,
            "biases": biases, "selt": selt, "et": et, "yb": yb,
            "m1m": m1m, "m1t": m1t, "m2": m2T,
        })
    return in_maps


def _assemble(results, y_seq, n_steps=T):
    """Host: reconstruct x_filt [B,T,S] and P_filt [B,T,S,S] from ens_v."""
    xs = np.zeros((B, n_steps, S), np.float32)
    Ps = np.zeros((B, n_steps, S, S), np.float32)
    for c in range(NCORES):
        ens = results[c]["ens"]                 # [T, S, JB]
        v = ens.reshape(n_steps, S, BL, NS).transpose(0, 2, 3, 1)
        mean_v = v.mean(axis=2)                 # [T, BL, S]
        diff = v - mean_v[:, :, None, :]        # [T, BL, NS, S]
        P = np.einsum("tbjs,tbju->tbsu", diff, diff,
                      dtype=np.float32) / NS
        x = np.zeros((BL, S), np.float32)
        for t in range(n_steps):
            x_pred = (0.9 * x + 0.1 * np.sin(x)).astype(np.float32)
            x = x_pred + mean_v[t]
            xs[c * BL:(c + 1) * BL, t] = x
            Ps[c * BL:(c + 1) * BL, t] = P[t]
    return xs, Ps


def _run(y_seq, W1, b1, W_ih, W_hh, b_ih, b_hh, W_out, b_out, num_samples,
         trace=False):
    import time as _time
    t0 = _time.time()
    nc = _build_nc()
    t1 = _time.time()
    in_maps = _host_inputs(y_seq, W1, b1, W_ih, W_hh, b_ih, b_hh, W_out,
                           b_out, num_samples)
    t2 = _time.time()
    res = run_bass_kernel_spmd(nc, in_maps, core_ids=list(range(NCORES)),
                               trace=trace)
    t3 = _time.time()
    xs, Ps = _assemble(res.results, y_seq)
    print(f"[kernel] build {t1-t0:.1f}s  host-prep {t2-t1:.1f}s  "
          f"device {t3-t2:.1f}s  assemble {_time.time()-t3:.1f}s")
    return (xs, Ps), res


def kernel(**inputs):
    (xs, Ps), _ = _run(**inputs)
    return xs, Ps


# revision 28
# speedup vs baseline: 1.1197x; 1.1197x over previous
"""BayesianKalmanNet Trainium2 kernel.

Sharding: data-parallel over batch (128 -> 16 per core x 8 cores); the
num_samples=16 ensemble stays on-core so the per-core GRU matmul free dim is
16*16 = 256.  Activations live transposed ([HID, b*NS+j]) so the GRU
recurrence needs no transposes.  Matmuls run as float32r (FP22), full-rate at
N=256.  Dropout masks are reproduced bit-exactly on the host (jax CPU
threefry) and shipped as uint8 with the 1/keep scale folded into weights /
activation scales.  The device emits only ens_v = K @ innov per timestep; the
host reconstructs x_filt and P_filt.

Schedule: the hidden state is double-buffered so h-updates retire per M-tile;
r/z/gi_n/n-combine/h-update are interleaved per M-tile; gh_n for step t+1 is
computed at the end of step t (it only needs h(t)) and drained to SBUF so the
PE has useful work covering the serial inter-step tail.
"""

import os
import functools

import numpy as np

os.environ.setdefault("JAX_COMPILATION_CACHE_DIR", "/tmp/jaxcache")
os.environ.setdefault("JAX_PERSISTENT_CACHE_MIN_COMPILE_TIME_SECS", "2")

import concourse.bass as bass
import concourse.tile as tile
from concourse import bacc, mybir
from concourse.bass import ts, ds
from concourse.bass_utils import run_bass_kernel_spmd

F32 = mybir.dt.float32
F32R = mybir.dt.float32r
I32 = mybir.dt.int32
U8 = mybir.dt.uint8
AL = mybir.AluOpType
AF = mybir.ActivationFunctionType

NCORES = 8
B, T, S, O = 128, 32, 8, 4
HID = 800
NS = 16
BL = B // NCORES          # 16 batches per core
JB = BL * NS              # 256 = free dim
SO = S * O                # 32
KEEP = 0.8
INV_KEEP = 1.25
MAGIC = 0x5F3759DF

KT_MAIN = 6
K_TAIL = HID - KT_MAIN * 128   # 32
M_TILES = [(i * 128, min(128, HID - i * 128)) for i in range(7)]

MM_PHASE = {}
RZ_BUFS = 3
GHN_BUFS = 2
SIN_POOL = False


def _r3(ap):
    """[P, 256] -> [P, 16, 16] (b outer, j inner)."""
    return ap.rearrange("p (b j) -> p b j", j=NS)


@functools.lru_cache(maxsize=4)
def _build_nc(n_steps=T, repeat=1):
    MM_PHASE.clear()
    nc = bacc.Bacc("TRN2", target_bir_lowering=False, debug=False,
                   num_devices=NCORES)

    d_wrz = nc.dram_tensor("wrz", [128, 13 * 1600], F32R, kind="ExternalInput")
    d_win = nc.dram_tensor("win", [128, 7 * HID], F32R, kind="ExternalInput")
    d_whn = nc.dram_tensor("whn", [128, 7 * HID], F32R, kind="ExternalInput")
    d_w1t = nc.dram_tensor("w1t", [17, HID], F32, kind="ExternalInput")
    d_wout = nc.dram_tensor("wout", [128, 7 * SO], F32R, kind="ExternalInput")
    d_bias = nc.dram_tensor("biases", [128, 36], F32, kind="ExternalInput")
    d_selt = nc.dram_tensor("selt", [SO, S], F32, kind="ExternalInput")
    d_et = nc.dram_tensor("et", [O, SO], F32, kind="ExternalInput")
    d_yb = nc.dram_tensor("yb", [BL, n_steps * O], F32, kind="ExternalInput")
    d_m1m = nc.dram_tensor("m1m", [n_steps, 128, KT_MAIN * JB], U8,
                           kind="ExternalInput")
    d_m1t = nc.dram_tensor("m1t", [n_steps, K_TAIL, JB], U8,
                           kind="ExternalInput")
    d_m2 = nc.dram_tensor("m2", [n_steps, SO, JB], U8, kind="ExternalInput")
    d_ens = nc.dram_tensor("ens", [n_steps, S, JB], F32, kind="ExternalOutput")

    def _mm(phase, *a, **k):
        bi = nc.tensor.matmul(*a, **k)
        MM_PHASE[bi.ins.name] = phase
        return bi

    with tile.TileContext(nc) as tc:
        with tc.tile_pool(name="const", bufs=1) as cpool, \
             tc.tile_pool(name="work", bufs=2) as wpool, \
             tc.tile_pool(name="ps", bufs=2, space="PSUM") as pspool:

            # ---- persistent tiles + weight loads ----
            bias_sb = cpool.tile([128, 36], F32, tag="bias")
            nc.sync.dma_start(bias_sb[:], d_bias.ap())
            selt_sb = cpool.tile([SO, S], F32, tag="selt")
            nc.sync.dma_start(selt_sb[:], d_selt.ap())
            et_sb = cpool.tile([O, SO], F32, tag="et")
            nc.sync.dma_start(et_sb[:], d_et.ap())
            w1t_sb = cpool.tile([17, HID], F32, tag="w1t")
            nc.sync.dma_start(w1t_sb[:], d_w1t.ap())
            yb_sb = cpool.tile([BL, n_steps * O], F32, tag="yb")
            nc.sync.dma_start(yb_sb[:], d_yb.ap())
            whn_sb = cpool.tile([128, 7 * HID], F32R, tag="whn")
            nc.sync.dma_start(whn_sb[:], d_whn.ap())
            wout_sb = cpool.tile([128, 7 * SO], F32R, tag="wout")
            nc.sync.dma_start(wout_sb[:], d_wout.ap())
            wrz_sb = cpool.tile([128, 13 * 1600], F32R, tag="wrz")
            nc.sync.dma_start(wrz_sb[:, 6 * 1600:], d_wrz.ap()[:, 6 * 1600:])
            nc.sync.dma_start(wrz_sb[:, 0:6 * 1600], d_wrz.ap()[:, 0:6 * 1600])
            win_sb = cpool.tile([128, 7 * HID], F32R, tag="win")
            nc.sync.dma_start(win_sb[:], d_win.ap())

            # double-buffered hidden state (read buf[t%2], write buf[1-t%2])
            # t6cat: rows 0:32 = h tail, rows 32:64 = a tail
            h_bufs, t6c_bufs = [], []
            for i in range(2):
                hb = cpool.tile([128, KT_MAIN * JB], F32R, tag=f"h{i}",
                                name=f"h{i}")
                nc.vector.memzero(hb[:])
                h_bufs.append(hb)
                tc_ = cpool.tile([2 * K_TAIL, JB], F32R, tag=f"t6c{i}",
                                 name=f"t6c{i}")
                nc.vector.memzero(tc_[:])
                t6c_bufs.append(tc_)

            tst_bufs, tstT_bufs, mst_bufs, mstT_bufs = [], [], [], []
            for i in range(2):
                tb_ = cpool.tile([32, 32], F32, tag=f"tstp{i}",
                                 name=f"tstp{i}")
                nc.vector.memset(tb_[:], 0.0)
                nc.vector.memset(tb_[0:BL, 16:17], 1.0)   # W1 bias row
                tst_bufs.append(tb_)
                tt_ = cpool.tile([32, 32], F32, tag=f"tstTp{i}",
                                 name=f"tstTp{i}")
                tstT_bufs.append(tt_)
                mb_ = cpool.tile([32, 32], F32, tag=f"mstp{i}",
                                 name=f"mstp{i}")
                nc.vector.memset(mb_[:], 0.0)
                mst_bufs.append(mb_)
                mt_ = cpool.tile([32, 32], F32, tag=f"mstTp{i}",
                                 name=f"mstTp{i}")
                mstT_bufs.append(mt_)

            xz = cpool.tile([BL, S], F32, tag="xz")
            nc.vector.memset(xz[:], 0.0)
            dz = cpool.tile([BL, S], F32, tag="dz")
            nc.vector.memset(dz[:], 0.0)
            scr8 = cpool.tile([BL, S], F32, tag="scr8")
            scr4 = cpool.tile([BL, O], F32, tag="scr4")

            def sm(tg, shape=(BL, S), dt=F32):
                return wpool.tile(list(shape), dt, tag=tg, name=tg)

            def emit_ghn(h_sb, t6c_sb):
                """gh_n for the next step: 49 MMs + ACT drain to SBUF."""
                kord_h = [(h_sb[:, ts(k, JB)], k) for k in range(KT_MAIN)]
                kord_h += [(t6c_sb[0:K_TAIL, :], 6)]
                tiles = []
                for mi, (mof, mp) in enumerate(M_TILES):
                    gh = pspool.tile([128, JB], F32, tag="ghn", name="ghnps",
                                     bufs=GHN_BUFS)
                    for ki, (rhs, col) in enumerate(kord_h):
                        _mm("gh_n", gh[0:mp, :],
                            whn_sb[0:rhs.partition_size(),
                                   ds(col * HID + mof, mp)],
                            rhs,
                            start=(ki == 0), stop=(ki == len(kord_h) - 1))
                    ghs = wpool.tile([128, JB], F32, tag="ghs", name="ghs",
                                     bufs=7)
                    nc.scalar.copy(ghs[0:mp, :], gh[0:mp, :])
                    tiles.append(ghs)
                return tiles

            x_prev, dx_prev = xz, dz
            ghs_next = emit_ghn(h_bufs[0], t6c_bufs[0])   # h == 0 prologue

            for step in range(n_steps * repeat):
                t = step % n_steps
                h_rd, h_wr = h_bufs[step % 2], h_bufs[1 - step % 2]
                t6c_rd, t6c_wr = t6c_bufs[step % 2], t6c_bufs[1 - step % 2]

                # ---- mask DMAs ----
                m1m = wpool.tile([128, KT_MAIN * JB], U8, tag="m1m",
                                 name="m1m")
                nc.sync.dma_start(m1m[:], d_m1m.ap()[t])
                m1t = wpool.tile([2 * K_TAIL, JB], U8, tag="m1t", name="m1t")
                nc.sync.dma_start(m1t[K_TAIL:2 * K_TAIL, :], d_m1t.ap()[t])
                m2t = wpool.tile([SO, JB], U8, tag="m2t", name="m2t")
                nc.sync.dma_start(m2t[:], d_m2.ap()[t])

                # ---- x_pred = 0.9 x + 0.1 sin(x) (deg-5 odd poly) ----
                se = nc.gpsimd if SIN_POOL else nc.vector
                tx = sm("tx")
                se.tensor_mul(tx[:], x_prev[:], x_prev[:])
                u1 = sm("u1")
                se.tensor_scalar(u1[:], tx[:], 1.0 / 120.0, -1.0 / 6.0,
                                 AL.mult, AL.add)
                v1 = sm("v1")
                se.tensor_mul(v1[:], u1[:], tx[:])
                sfac = sm("sfac")
                # sin(x)/x ~= 1 + v1 ; x_pred = x*(0.9 + 0.1*(1 + v1))
                se.tensor_scalar(sfac[:], v1[:], 0.1, 1.0,
                                 AL.mult, AL.add)
                x_pred = sm("x_pred")
                se.tensor_mul(x_pred[:], x_prev[:], sfac[:])

                # ---- innov into tst[:, 0:4] ----
                yhat = sm("yhat", (BL, O))
                nc.scalar.activation(yhat[:], x_pred[:, 0:O], AF.Tanh)
                tst = tst_bufs[step % 2]
                nc.vector.tensor_sub(tst[0:BL, 0:O], yb_sb[:, ts(t, O)],
                                     yhat[:])

                # ---- l2 norms on DVE; inv = rsqrt via quake + 1 Newton ----
                ss2 = sm("ss2", (BL, 2))
                nc.vector.scalar_tensor_tensor(
                    scr8[:], dx_prev[:], 1.0, dx_prev[:], AL.mult, AL.mult,
                    accum_out=ss2[:, 0:1])
                nc.vector.scalar_tensor_tensor(
                    scr4[:], tst[0:BL, 0:O], 1.0, tst[0:BL, 0:O],
                    AL.mult, AL.mult, accum_out=ss2[:, 1:2])
                sscl = sm("sscl", (BL, 2))
                nc.vector.tensor_scalar_max(sscl[:], ss2[:], 1e-24)
                qi = sm("qi", (BL, 2), I32)
                nc.vector.tensor_scalar(qi[:], sscl[:].bitcast(I32), 1, None,
                                        AL.logical_shift_right)
                qi2 = sm("qi2", (BL, 2), I32)
                nc.vector.tensor_scalar(qi2[:], qi[:], -1, MAGIC,
                                        AL.mult, AL.add)
                yv = qi2[:].bitcast(F32)
                for it in range(2):
                    nw1 = sm(f"nw1_{it}", (BL, 2))
                    nc.vector.tensor_mul(nw1[:], yv, yv)
                    nw2 = sm(f"nw2_{it}", (BL, 2))
                    nc.vector.tensor_mul(nw2[:], nw1[:], sscl[:])
                    nw3 = sm(f"nw3_{it}", (BL, 2))
                    nc.vector.tensor_scalar(nw3[:], nw2[:], -0.5, 1.5,
                                            AL.mult, AL.add)
                    ny = sm(f"ny_{it}", (BL, 2))
                    nc.vector.tensor_mul(ny[:], yv, nw3[:])
                    yv = ny[:]
                nc.vector.tensor_mul(tst[0:BL, 4:12], dx_prev[:],
                                     yv[:, 0:1].to_broadcast([BL, S]))
                nc.vector.tensor_mul(tst[0:BL, 12:16], tst[0:BL, 0:O],
                                     yv[:, 1:2].to_broadcast([BL, O]))
                tstT = tstT_bufs[step % 2]
                nc.vector.transpose(tstT[:], tst[:])

                # innov broadcast for the ens contraction (used late)
                ibc = wpool.tile([O, JB], F32, tag="ibc", name="ibc")
                nc.vector.tensor_copy(
                    _r3(ibc[:]),
                    tstT[0:O, 0:BL, None].to_broadcast([O, BL, NS]))

                # ---- a = relu(1.25*(W1 @ nn_in + b1)) (transposed) ----
                aT = wpool.tile([128, 7 * BL], F32, tag="aT", name="aT")
                aps = pspool.tile([128, 7 * BL], F32, tag="misc", name="aps", bufs=1)
                for mi, (mof, mp) in enumerate(M_TILES):
                    if mi < KT_MAIN:
                        out_ap = aps[0:mp, ts(mi, BL)]
                    else:
                        out_ap = aps[K_TAIL:2 * K_TAIL, ts(mi, BL)]
                    _mm("a_w1", out_ap,
                        w1t_sb[0:17, ds(mof, mp)], tstT[0:17, 0:BL],
                        start=True, stop=True)
                nc.scalar.activation(aT[:, 0:BL], aps[:, 0:BL], AF.Relu,
                                     scale=INV_KEEP)
                nc.scalar.activation(aT[:, BL:KT_MAIN * BL],
                                     aps[:, BL:KT_MAIN * BL], AF.Relu,
                                     scale=INV_KEEP)
                nc.scalar.activation(aT[K_TAIL:2 * K_TAIL, ds(KT_MAIN * BL, BL)],
                                     aps[K_TAIL:2 * K_TAIL, ds(KT_MAIN * BL, BL)],
                                     AF.Relu, scale=INV_KEEP)

                # ---- a_ens = aT (bcast over j) * m1 ----
                aens = wpool.tile([128, KT_MAIN * JB], F32R, tag="aens",
                                  name="aens")
                for k in range(KT_MAIN):
                    nc.vector.tensor_mul(
                        _r3(aens[:, ts(k, JB)]),
                        aT[:, ts(k, BL), None].to_broadcast([128, BL, NS]),
                        _r3(m1m[:, ts(k, JB)]))
                nc.vector.tensor_mul(
                    _r3(t6c_rd[K_TAIL:2 * K_TAIL, :]),
                    aT[K_TAIL:2 * K_TAIL, ds(6 * BL, BL), None].to_broadcast(
                        [K_TAIL, BL, NS]),
                    _r3(m1t[K_TAIL:2 * K_TAIL, :]))

                # ---- per-M-tile: r, z, gi_n, n-combine, h-update ----
                korder = [(h_rd[:, ts(k, JB)], 6 + k) for k in range(KT_MAIN)]
                korder += [(aens[:, ts(k, JB)], k) for k in range(KT_MAIN)]
                korder += [(t6c_rd[0:2 * K_TAIL, :], 12)]
                kord_a = [(aens[:, ts(k, JB)], k) for k in range(KT_MAIN)]
                kord_a += [(t6c_rd[K_TAIL:2 * K_TAIL, :], 6)]
                ghs_cur = ghs_next

                for mi, (mof, mp) in enumerate(M_TILES):
                    gate_sb = {}
                    for g, gname, bcol0 in ((0, "rT", 0), (1, "zT", 7)):
                        mbase = g * HID + mof
                        ps = pspool.tile([128, JB], F32, tag="rz",
                                         name="rzps", bufs=RZ_BUFS)
                        for ki, (rhs, col) in enumerate(korder):
                            _mm("rz_h" if ki < 7 else "rz_a",
                                ps[0:mp, :],
                                wrz_sb[0:rhs.partition_size(),
                                       ds(col * 1600 + mbase, mp)],
                                rhs,
                                start=(ki == 0), stop=(ki == len(korder) - 1))
                        gt = wpool.tile([128, JB], F32, tag=gname, name=gname,
                                        bufs=3)
                        nc.scalar.activation(
                            gt[0:mp, :], ps[0:mp, :], AF.Sigmoid,
                            bias=bias_sb[0:mp, bcol0 + mi:bcol0 + mi + 1])
                        gate_sb[gname] = gt

                    gi = pspool.tile([128, JB], F32, tag="gin", name="ginps")
                    for ki, (rhs, col) in enumerate(kord_a):
                        bp = rhs.base_partition()
                        _mm("gi_n", gi[0:mp, :],
                            win_sb[bp:bp + rhs.partition_size(),
                                   ds(col * HID + mof, mp)],
                            rhs,
                            start=(ki == 0), stop=(ki == len(kord_a) - 1))
                    tb = wpool.tile([128, JB], F32, tag="tb", name="tb")
                    nc.vector.scalar_tensor_tensor(
                        tb[0:mp, :], ghs_cur[mi][0:mp, :],
                        bias_sb[0:mp, 21 + mi:22 + mi],
                        gate_sb["rT"][0:mp, :], AL.add, AL.mult)
                    npre = wpool.tile([128, JB], F32, tag="npre", name="npre")
                    nc.vector.scalar_tensor_tensor(
                        npre[0:mp, :], gi[0:mp, :],
                        bias_sb[0:mp, 14 + mi:15 + mi], tb[0:mp, :],
                        AL.add, AL.add)
                    nT = wpool.tile([128, JB], F32, tag="nT", name="nT",
                                    bufs=3)
                    nc.scalar.activation(nT[0:mp, :], npre[0:mp, :], AF.Tanh)

                    # h_new = n + z * (h - n) into the write buffer
                    hsrc = (h_rd[:, ts(mi, JB)] if mi < KT_MAIN
                            else t6c_rd[0:K_TAIL, :])
                    htgt = (h_wr[:, ts(mi, JB)] if mi < KT_MAIN
                            else t6c_wr[0:K_TAIL, :])
                    qb = wpool.tile([128, JB], F32, tag="qb", name="qb")
                    nc.vector.scalar_tensor_tensor(
                        qb[0:mp, :], nT[0:mp, :], -1.0, hsrc,
                        AL.mult, AL.add)
                    eb = wpool.tile([128, JB], F32, tag="eb", name="eb")
                    nc.vector.tensor_mul(eb[0:mp, :],
                                         gate_sb["zT"][0:mp, :], qb[0:mp, :])
                    nc.vector.tensor_add(htgt, nT[0:mp, :], eb[0:mp, :])

                # ---- K_vec = 1.25*(W_out @ h_new + b_out) * m2 ----
                kord_hw = [(h_wr[:, ts(k, JB)], k) for k in range(KT_MAIN)]
                kord_hw += [(t6c_wr[0:K_TAIL, :], 6)]
                kv = pspool.tile([SO, JB], F32, tag="misc", name="kvps", bufs=1)
                for ki, (rhs, col) in enumerate(kord_hw):
                    _mm("kvec", kv[:, :],
                        wout_sb[0:rhs.partition_size(), ds(col * SO, SO)],
                        rhs,
                        start=(ki == 0), stop=(ki == len(kord_hw) - 1))
                KT = wpool.tile([SO, JB], F32, tag="KT", name="KT")
                nc.vector.scalar_tensor_tensor(
                    KT[:], kv[:, :], bias_sb[0:SO, 35:36], m2t[:],
                    AL.add, AL.mult)

                # ---- ens_v = Sel @ (K .* innov_exp) ----
                iexp = pspool.tile([SO, JB], F32, tag="misc", name="iexpps", bufs=1)
                _mm("iexp", iexp[:, :], et_sb[:], ibc[:],
                    start=True, stop=True)
                prod = wpool.tile([SO, JB], F32, tag="prod", name="prod")
                nc.vector.tensor_mul(prod[:], KT[:], iexp[:, :])
                ensps = pspool.tile([S, JB], F32, tag="misc", name="ensps", bufs=1)
                _mm("ens", ensps[:, :], selt_sb[:], prod[:],
                    start=True, stop=True)
                enssb = wpool.tile([S, JB], F32, tag="enssb", name="enssb")
                nc.scalar.copy(enssb[:], ensps[:, :])
                nc.sync.dma_start(d_ens.ap()[t], enssb[:])

                # ---- x_filt = x_pred + mean_j(ens_v) ----
                mtmp = sm("mtmp", (S, BL))
                nc.vector.tensor_reduce(mtmp[:], _r3(ensps[:, :]),
                                        mybir.AxisListType.X, AL.add)
                mst = mst_bufs[step % 2]
                nc.vector.tensor_scalar_mul(mst[0:S, 0:BL], mtmp[:],
                                            1.0 / NS)
                mstT = mstT_bufs[step % 2]
                nc.vector.transpose(mstT[:], mst[:])
                xfb = sm("xfb")
                nc.vector.tensor_add(xfb[:], x_pred[:], mstT[0:BL, 0:S])

                # gh_n for next step (gives PE work covering the serial tail)
                ghs_next = emit_ghn(h_wr, t6c_wr)

                x_prev = xfb
                dx_prev = mstT[0:BL, 0:S]

    nc.compile()
    return nc


def _host_inputs(y_seq, W1, b1, W_ih, W_hh, b_ih, b_hh, W_out, b_out,
                 num_samples, n_steps=T):
    """Build the 8 per-core input maps."""
    import jax

    assert int(num_samples) == NS
    y_seq = np.asarray(y_seq, np.float32)
    W1 = np.asarray(W1, np.float32)
    b1 = np.asarray(b1, np.float32)
    W_ih = np.asarray(W_ih, np.float32)
    W_hh = np.asarray(W_hh, np.float32)
    b_ih = np.asarray(b_ih, np.float32)
    b_hh = np.asarray(b_hh, np.float32)
    W_out = np.asarray(W_out, np.float32)
    b_out = np.asarray(b_out, np.float32)

    WihT = np.ascontiguousarray(W_ih.T)   # [HID, 2400]
    WhhT = np.ascontiguousarray(W_hh.T)

    wrz = np.zeros((128, 13 * 1600), np.float32)
    for k in range(6):
        wrz[:, k * 1600:(k + 1) * 1600] = WihT[k * 128:(k + 1) * 128, 0:1600]
        wrz[:, (6 + k) * 1600:(7 + k) * 1600] = \
            WhhT[k * 128:(k + 1) * 128, 0:1600]
    # merged tail block: rows 0:32 = W_hh tail (h), rows 32:64 = W_ih tail (a)
    wrz[0:K_TAIL, 12 * 1600:] = WhhT[768:HID, 0:1600]
    wrz[K_TAIL:2 * K_TAIL, 12 * 1600:] = WihT[768:HID, 0:1600]

    def pack_n(WT):
        w = np.zeros((128, 7 * HID), np.float32)
        for k in range(6):
            w[:, k * HID:(k + 1) * HID] = WT[k * 128:(k + 1) * 128, 1600:2400]
        w[0:K_TAIL, 6 * HID:] = WT[768:HID, 1600:2400]
        return w

    win = pack_n(WihT)
    win[K_TAIL:2 * K_TAIL, 6 * HID:] = win[0:K_TAIL, 6 * HID:]
    win[0:K_TAIL, 6 * HID:] = 0.0
    whn = pack_n(WhhT)

    w1t = np.zeros((17, HID), np.float32)   # rows 0:4 zero (raw innov)
    w1t[4:16] = W1.T                        # dxn rows 4:12, innovn 12:16
    w1t[16] = b1                            # bias via ones row of nn_inT
    WoutTs = np.ascontiguousarray(W_out.T) * INV_KEEP      # [800, 32]
    wout = np.zeros((128, 7 * SO), np.float32)
    for k in range(6):
        wout[:, k * SO:(k + 1) * SO] = WoutTs[k * 128:(k + 1) * 128]
    wout[0:K_TAIL, 6 * SO:] = WoutTs[768:HID]

    biases = np.zeros((128, 36), np.float32)
    b_rz = b_ih[0:1600] + b_hh[0:1600]
    for mi, (mof, mp) in enumerate(M_TILES):
        biases[0:mp, mi] = b_rz[mof:mof + mp]                    # r
        biases[0:mp, 7 + mi] = b_rz[HID + mof:HID + mof + mp]    # z
        biases[0:mp, 14 + mi] = b_ih[1600 + mof:1600 + mof + mp]
        biases[0:mp, 21 + mi] = b_hh[1600 + mof:1600 + mof + mp]
        biases[0:mp, 28 + mi] = b1[mof:mof + mp] * INV_KEEP
    biases[0:SO, 35] = b_out * INV_KEEP

    selt = np.zeros((SO, S), np.float32)
    for s in range(S):
        selt[s * O:(s + 1) * O, s] = 1.0
    et = np.zeros((O, SO), np.float32)
    for s in range(S):
        for o in range(O):
            et[o, s * O + o] = 1.0

    # dropout masks (bit-exact threefry reproduction of the reference)
    import jax.random as jr
    cpu = jax.devices("cpu")[0]
    m1_all = np.empty((n_steps, NS, B, HID), np.uint8)
    m2_all = np.empty((n_steps, NS, B, SO), np.uint8)
    with jax.default_device(cpu):
        drop_key = jr.key(42)
        for t in range(n_steps):
            k1, k2 = jr.split(jr.fold_in(drop_key, t))
            m1_all[t] = np.asarray(jr.bernoulli(k1, KEEP, (NS, B, HID)),
                                   np.uint8)
            m2_all[t] = np.asarray(jr.bernoulli(k2, KEEP, (NS, B, SO)),
                                   np.uint8)

    in_maps = []
    for c in range(NCORES):
        bs = slice(c * BL, (c + 1) * BL)
        m1T = m1_all[:, :, bs, :].transpose(0, 3, 2, 1).reshape(
            n_steps, HID, JB)
        m1m = np.ascontiguousarray(
            m1T[:, 0:768, :].reshape(n_steps, 6, 128, JB)
            .transpose(0, 2, 1, 3).reshape(n_steps, 128, 6 * JB))
        m1t = np.ascontiguousarray(m1T[:, 768:HID, :])
        m2T = np.ascontiguousarray(
            m2_all[:, :, bs, :].transpose(0, 3, 2, 1).reshape(
                n_steps, SO, JB))
        yb = np.ascontiguousarray(
            y_seq[bs, 0:n_steps].reshape(BL, n_steps * O))
        in_maps.append({
            "wrz": wrz, "win": win, "whn": whn, "w1t": w1t, "wout": wout,
            "biases": biases, "selt": selt, "et": et, "yb": yb,
            "m1m": m1m, "m1t": m1t, "m2": m2T,
        })
    return in_maps


def _assemble(results, y_seq, n_steps=T):
    """Host: reconstruct x_filt [B,T,S] and P_filt [B,T,S,S] from ens_v."""
    xs = np.zeros((B, n_steps, S), np.float32)
    Ps = np.zeros((B, n_steps, S, S), np.float32)
    for c in range(NCORES):
        ens = results[c]["ens"]                 # [T, S, JB]
        v = ens.reshape(n_steps, S, BL, NS).transpose(0, 2, 3, 1)
        mean_v = v.mean(axis=2)                 # [T, BL, S]
        diff = v - mean_v[:, :, None, :]        # [T, BL, NS, S]
        P = np.einsum("tbjs,tbju->tbsu", diff, diff,
                      dtype=np.float32) / NS
        x = np.zeros((BL, S), np.float32)
        for t in range(n_steps):
            x_pred = (0.9 * x + 0.1 * np.sin(x)).astype(np.float32)
            x = x_pred + mean_v[t]
            xs[c * BL:(c + 1) * BL, t] = x
            Ps[c * BL:(c + 1) * BL, t] = P[t]
    return xs, Ps


def _run(y_seq, W1, b1, W_ih, W_hh, b_ih, b_hh, W_out, b_out, num_samples,
         trace=False):
    import time as _time
    t0 = _time.time()
    nc = _build_nc()
    t1 = _time.time()
    in_maps = _host_inputs(y_seq, W1, b1, W_ih, W_hh, b_ih, b_hh, W_out,
                           b_out, num_samples)
    t2 = _time.time()
    res = run_bass_kernel_spmd(nc, in_maps, core_ids=list(range(NCORES)),
                               trace=trace)
    t3 = _time.time()
    xs, Ps = _assemble(res.results, y_seq)
    print(f"[kernel] build {t1-t0:.1f}s  host-prep {t2-t1:.1f}s  "
          f"device {t3-t2:.1f}s  assemble {_time.time()-t3:.1f}s")
    return (xs, Ps), res


def kernel(**inputs):
    (xs, Ps), _ = _run(**inputs)
    return xs, Ps
